# revision 1
# baseline (speedup 1.0000x reference)
"""Trainium2 Bass kernel for nn_Encoder (4-layer dense transformer encoder).

Sharding: sequence-sharded data parallel. 8 cores = 2 batches x 4 sequence
chunks of 256 tokens. Per layer each core computes its own K chunk (both the
[HD, keys] and [keys, HD] layouts) and AllGathers K within its batch's
4-core group. Activations stay transposed (xT [D, L_local]) so every matmul
contracts over the partition dim. All matmul inputs are float32r (fp32 bits;
the PE rounds to 11 explicit mantissa bits and runs at full rate for N>=256).

The attention key mask is applied multiplicatively: rows of k_nat for masked
keys are zeroed and the softmax denominator uses the 0/1 keep-mask as the
stationary column, which is numerically identical to the reference's
where(mask, -1e9, score) followed by softmax.

Self-contained: hardcodes all shapes; host side does the embedding gather,
positional encoding, weight permutations/reshapes, and output assembly.
"""
import os
import numpy as np

B, L, D, H, M, NL, V = 2, 1024, 1024, 16, 4096, 4, 32000
HD = D // H          # 64
LC = 256             # local sequence chunk per core
NCORES = 8
PAD = 0

_DEV_NL = int(os.environ.get("KERNEL_NL", str(NL)))

_cache = {}
PHASE_LOG = []


def _build_nc(n_layers, reps=1):
    import os
    import contextlib
    import concourse.mybir as mybir
    import concourse.tile as tile
    from concourse import bacc
    from concourse.masks import make_identity

    f32 = mybir.dt.float32
    f32r = mybir.dt.float32r
    AF = mybir.ActivationFunctionType
    ALU = mybir.AluOpType

    FAKE_AG = os.environ.get("KERNEL_FAKE_AG", "0") == "1"
    nc = bacc.Bacc(None, target_bir_lowering=False, num_devices=NCORES)
    PHASE_LOG.clear()

    def mark(label):
        PHASE_LOG.append((label, len(nc.inst_map)))

    def par(name, shape, dt, out=False):
        return nc.declare_dram_parameter(name, list(shape), dt, isOutput=out)

    xT_in = par("xT_in", [8, 128, LC], f32r)
    wq_in = par("wq", [n_layers, 8, 128, 1024], f32r)   # [li][di][p][head-major col]
    wk_in = par("wk", [n_layers, 8, 128, 1024], f32r)
    wo_in = par("wo", [n_layers, 8, 128, 1024], f32r)   # [li][hp][p][e]
    w1_in = par("w1", [n_layers, 4, 8, 128, 8, 128], f32r)  # [li][jg][di][p][jj][q]
    w2_in = par("w2", [n_layers, 32, 128, 1024], f32r)  # [li][jc][p][e]
    b1c_in = par("b1c", [n_layers, 32, 128], f32)
    b2c_in = par("b2c", [n_layers, 8, 128], f32)
    g1r_in = par("g1r", [n_layers, 8, 128], f32r)
    b1r_in = par("b1r", [n_layers, 8, 128], f32r)
    g2r_in = par("g2r", [n_layers, 8, 128], f32r)
    b2r_in = par("b2r", [n_layers, 8, 128], f32r)
    km_in = par("km", [8, 128], f32r)                   # 0/1 keep-mask, all 1024 keys
    kmo_in = par("kmo", [2, 128], f32r)                 # keep-mask for own 256 keys
    ones_in = par("ones", [128, 256], f32r)
    out_par = par("out", [LC, D], f32, out=True)
    DBG = os.environ.get("KERNEL_DEBUG", "0") == "1"
    if DBG:
        dbg_q = par("dbg_q", [128, 8, 256], f32, out=True)
        dbg_kt = par("dbg_kt", [128, 8, 1024], f32, out=True)
        dbg_kn = par("dbg_kn", [128, 8, 1024], f32, out=True)
        dbg_w0 = par("dbg_w0", [128, 8, 256], f32, out=True)
        dbg_at = par("dbg_at", [128, 8, 256], f32, out=True)
        dbg_r1 = par("dbg_r1", [128, 8, 256], f32, out=True)
        dbg_x1 = par("dbg_x1", [128, 8, 256], f32, out=True)

    EPS = 1e-5
    INV_D = 1.0 / float(D)
    INV_SQ = 0.125  # 1/sqrt(HD)

    with tile.TileContext(nc) as tc:
        ctx = contextlib.ExitStack()
        with ctx:
            sbc = ctx.enter_context(tc.tile_pool(name="const", bufs=1))
            sbx = ctx.enter_context(tc.tile_pool(name="xt", bufs=2))
            sbk = ctx.enter_context(tc.tile_pool(name="kbuf", bufs=1))
            sbq = ctx.enter_context(tc.tile_pool(name="qbuf", bufs=1))
            sbw = ctx.enter_context(tc.tile_pool(name="wts", bufs=4))
            sba = ctx.enter_context(tc.tile_pool(name="act", bufs=2))
            sbh = ctx.enter_context(tc.tile_pool(name="hbuf", bufs=1))
            sbs = ctx.enter_context(tc.tile_pool(name="small", bufs=4))
            psp = ctx.enter_context(tc.tile_pool(name="ps", bufs=2, space="PSUM"))
            drp = ctx.enter_context(tc.tile_pool(name="dram", bufs=2, space="DRAM"))

            ones = sbc.tile([128, 256], f32r, name="ones_t")
            nc.sync.dma_start(out=ones[:, :], in_=ones_in[:, :])
            km = sbc.tile([128, 8], f32r, name="km_t")
            nc.sync.dma_start(out=km[:, :], in_=km_in.rearrange("m p -> p m"))
            kmo = sbc.tile([128, 2], f32r, name="kmo_t")
            nc.sync.dma_start(out=kmo[:, :], in_=kmo_in.rearrange("m p -> p m"))
            ident = sbc.tile([128, 128], f32, name="ident_t")
            make_identity(nc, ident[:, :])

            xT = sbx.tile([128, 8, LC], f32r, tag="xT", name="xT0")
            nc.sync.dma_start(out=xT[:, :, :], in_=xT_in.rearrange("e p l -> p e l"))

            def layer_norm(resid, xT_out, gr_dram, br_dram, uid):
                """xT_out = LN(resid) * g + b, all per-column-l stats."""
                grt = sbs.tile([1, 8, 128], f32r, tag="gr", bufs=1, name=f"gr_{uid}")
                nc.sync.dma_start(out=grt[:, :, :], in_=gr_dram.unsqueeze(0))
                brt = sbs.tile([1, 8, 128], f32r, tag="br", bufs=1, name=f"br_{uid}")
                nc.sync.dma_start(out=brt[:, :, :], in_=br_dram.unsqueeze(0))

                ps_st = psp.tile([128, 2048], f32, tag="ps", name=f"psst_{uid}")
                for ei in range(8):
                    st, sp = ei == 0, ei == 7
                    sq1 = sbs.tile([128, 256], f32r, tag="sqtmp", bufs=2,
                                   name=f"sq_{uid}_{ei}")
                    nc.scalar.activation(sq1[:, :], resid[:, ei, :].bitcast(f32),
                                         AF.Square)
                    nc.tensor.matmul(ps_st[0:1, 0:256], ones[:, 0:1],
                                     resid[:, ei, :], start=st, stop=sp)
                    nc.tensor.matmul(ps_st[0:1, 256:512], ones[:, 0:1],
                                     sq1[:, :], start=False, stop=sp)
                mu = sbs.tile([1, 256], f32, tag="st1", bufs=1, name=f"mu_{uid}")
                nc.vector.tensor_scalar_mul(mu[:, :], ps_st[0:1, 0:256], INV_D)
                ex2 = sbs.tile([1, 256], f32, tag="st2", bufs=1, name=f"ex2_{uid}")
                nc.vector.tensor_scalar_mul(ex2[:, :], ps_st[0:1, 256:512], INV_D)
                mu2 = sbs.tile([1, 256], f32, tag="st3", bufs=1, name=f"mu2_{uid}")
                nc.vector.tensor_mul(mu2[:, :], mu[:, :], mu[:, :])
                var = sbs.tile([1, 256], f32, tag="st4", bufs=1, name=f"var_{uid}")
                nc.vector.scalar_tensor_tensor(
                    out=var[:, :], in0=ex2[:, :], scalar=EPS, in1=mu2[:, :],
                    op0=ALU.add, op1=ALU.subtract)
                sd = sbs.tile([1, 256], f32, tag="st5", bufs=1, name=f"sd_{uid}")
                nc.scalar.activation(sd[:, :], var[:, :], AF.Sqrt)
                rstd = sbs.tile([1, 256], f32r, tag="st6", bufs=1, name=f"rstd_{uid}")
                with nc.allow_low_precision(reason="f32r rounding ok"):
                    nc.vector.reciprocal(rstd[:, :], sd[:, :])
                nmr = sbs.tile([1, 256], f32r, tag="st7", bufs=1, name=f"nmr_{uid}")
                nc.vector.scalar_tensor_tensor(
                    out=nmr[:, :], in0=mu[:, :], scalar=-1.0, in1=rstd[:, :].bitcast(f32),
                    op0=ALU.mult, op1=ALU.mult)
                ps_rg = psp.tile([128, 2048], f32, tag="ps", name=f"psrg_{uid}")
                ps_nb = psp.tile([128, 2048], f32, tag="ps", name=f"psnb_{uid}")
                for ei in range(8):
                    nc.tensor.matmul(ps_rg[:, ei * 256:(ei + 1) * 256],
                                     grt[0:1, ei, :], rstd[:, :],
                                     start=True, stop=True)
                    nc.tensor.matmul(ps_nb[:, ei * 256:(ei + 1) * 256],
                                     grt[0:1, ei, :], nmr[:, :],
                                     start=True, stop=False)
                    nc.tensor.matmul(ps_nb[:, ei * 256:(ei + 1) * 256],
                                     brt[0:1, ei, :], ones[0:1, 0:256],
                                     start=False, stop=True)
                for ei in range(8):
                    tmp = sbs.tile([128, 256], f32, tag="lntmp", name=f"lnt_{uid}_{ei}", bufs=2)
                    nc.vector.tensor_mul(tmp[:, :], resid[:, ei, :].bitcast(f32),
                                         ps_rg[:, ei * 256:(ei + 1) * 256])
                    nc.vector.tensor_tensor(
                        out=xT_out[:, ei, :], in0=tmp[:, :],
                        in1=ps_nb[:, ei * 256:(ei + 1) * 256], op=ALU.add)

            for rep in range(reps):
              for li in range(n_layers):
                  # ---------------- K phase: own-chunk kT and k_nat ----------
                  mark(f"L{li}.k")
                  ps_kt = psp.tile([128, 2048], f32, tag="ps", name=f"pskt_{li}")
                  ps_kn = psp.tile([128, 2048], f32, tag="ps", name=f"pskn_{li}")
                  for di in range(8):
                      wkt = sbw.tile([128, 1024], f32r, tag="w", name=f"wk_{li}_{di}")
                      nc.sync.dma_start(out=wkt[:, :], in_=wk_in[li, di])
                      st, sp = di == 0, di == 7
                      for hp in range(8):
                          nc.tensor.matmul(
                              ps_kt[:, hp * 256:(hp + 1) * 256],
                              wkt[:, hp * 128:(hp + 1) * 128],
                              xT[:, di, :], start=st and hp % 2 == 0, stop=sp)
                      for ms in range(2):
                          for nn_ in range(2):
                              nc.tensor.matmul(
                                  ps_kn[:, ms * 1024 + nn_ * 512:ms * 1024 + (nn_ + 1) * 512],
                                  xT[:, di, ms * 128:(ms + 1) * 128],
                                  wkt[:, nn_ * 512:(nn_ + 1) * 512], start=st, stop=sp)
                  kTc = sbs.tile([128, 2048], f32r, tag="kstage", bufs=1,
                                 name=f"kTc_{li}")
                  nc.vector.tensor_copy(kTc[:, :], ps_kt[:, :])
                  knc = sbs.tile([128, 2048], f32r, tag="kstage", bufs=1,
                                 name=f"knc_{li}")
                  for ms in range(2):
                      # zero masked keys (keys are partitions here) of own chunk
                      nc.vector.tensor_scalar_mul(
                          knc[:, ms * 1024:(ms + 1) * 1024],
                          ps_kn[:, ms * 1024:(ms + 1) * 1024],
                          kmo[:, ms:ms + 1].bitcast(f32))

                  # one merged AllGather per layer: blocks 0-7 = kT chunk,
                  # blocks 8-15 = k_nat chunk
                  agi = drp.tile([16, 128, 256], f32r, tag="agi", name=f"agi_{li}")
                  ago = drp.tile([4, 16, 128, 256], f32r, tag="ago", name=f"ago_{li}")
                  nc.sync.dma_start(out=agi[0:8].rearrange("h p m -> p h m"),
                                    in_=kTc[:, :].rearrange("p (h m) -> p h m", h=8))
                  nc.sync.dma_start(out=agi[8:16].rearrange("b p c -> p b c"),
                                    in_=knc[:, :].rearrange("p (b c) -> p b c", b=8))
                  if FAKE_AG:
                      for r in range(4):
                          nc.sync.dma_start(out=ago[r], in_=agi[:, :, :])
                  else:
                      nc.gpsimd.collective_compute(
                          "AllGather", mybir.AluOpType.bypass,
                          replica_groups=[[0, 1, 2, 3], [4, 5, 6, 7]],
                          ins=[agi.opt()], outs=[ago.opt()])

                  mark(f"L{li}.q")
                  # ---------------- Q phase (overlaps AG) --------------------
                  ps_q = psp.tile([128, 2048], f32, tag="ps", name=f"psq_{li}")
                  for di in range(8):
                      wqt = sbw.tile([128, 1024], f32r, tag="w", name=f"wq_{li}_{di}")
                      nc.sync.dma_start(out=wqt[:, :], in_=wq_in[li, di])
                      for hp in range(8):
                          nc.tensor.matmul(
                              ps_q[:, hp * 256:(hp + 1) * 256],
                              wqt[:, hp * 128:(hp + 1) * 128],
                              xT[:, di, :], start=di == 0 and hp % 2 == 0,
                              stop=di == 7)
                  qT = sbq.tile([128, 8, 256], f32r, tag="qT", name=f"qT_{li}")
                  nc.vector.tensor_copy(qT[:, :, :],
                                        ps_q[:, :].rearrange("p (h m) -> p h m", h=8))

                  kT = sbk.tile([128, 8, 1024], f32r, tag="kT", name=f"kT_{li}")
                  kn = sbk.tile([128, 8, 1024], f32r, tag="kn", name=f"kn_{li}")
                  for r in range(4):
                      nc.sync.dma_start(
                          out=kT[:, :, r * 256:(r + 1) * 256],
                          in_=ago[r, 0:8].rearrange("h p m -> p h m"))
                      nc.sync.dma_start(
                          out=kn[:, r * 2:(r + 1) * 2, :].rearrange("p s r2 -> p (s r2)").rearrange("p (b c) -> p b c", b=8),
                          in_=ago[r, 8:16].rearrange("b p c -> p b c"))
                  if DBG and li == 0:
                      nc.sync.dma_start(out=dbg_q.rearrange("p h m -> p h m"),
                                        in_=qT[:, :, :].bitcast(f32))
                      nc.sync.dma_start(out=dbg_kt[:, :, :], in_=kT[:, :, :].bitcast(f32))
                      nc.sync.dma_start(out=dbg_kn[:, :, :], in_=kn[:, :, :].bitcast(f32))

                  mark(f"L{li}.attn")
                  # ---------------- attention, head by head ------------------
                  attnT = sba.tile([128, 8, 256], f32r, tag="attnT", bufs=1, name=f"attnT_{li}")
                  for h in range(H):
                      hp, sub = h // 2, h % 2
                      lo, hi = sub * 64, sub * 64 + 64
                      ps_s = psp.tile([128, 2048], f32, tag="ps", name=f"pss_{li}_{h}")
                      for mi in range(8):
                          nc.tensor.matmul(
                              ps_s[:, mi * 256:(mi + 1) * 256],
                              kT[lo:hi, hp, mi * 128:(mi + 1) * 128],
                              qT[lo:hi, hp, :], start=True, stop=True)
                      wT = sba.tile([128, 8, 256], f32r, tag="wT", name=f"wT_{li}_{h}")
                      for w_ in range(2):
                          nc.scalar.activation(
                              wT[:, w_ * 4:(w_ + 1) * 4, :],
                              ps_s[:, w_ * 1024:(w_ + 1) * 1024].rearrange(
                                  "p (i m) -> p i m", i=4),
                              AF.Exp, bias=0.0, scale=INV_SQ)
                      if DBG and li == 0 and h == 0:
                          nc.sync.dma_start(out=dbg_w0[:, :, :], in_=wT[:, :, :].bitcast(f32))
                      ps_o = psp.tile([128, 2048], f32, tag="ps", name=f"pso_{li}_{h}")
                      for mi in range(8):
                          st, sp = mi == 0, mi == 7
                          nc.tensor.matmul(
                              ps_o[0:64, 0:256],
                              kn[:, mi, h * 64:(h + 1) * 64],
                              wT[:, mi, :], start=st, stop=sp)
                          nc.tensor.matmul(
                              ps_o[0:1, 256:512],
                              km[:, mi:mi + 1],
                              wT[:, mi, :], start=False, stop=sp)
                      rcp = sbs.tile([1, 256], f32r, tag="rcp", bufs=2, name=f"rcp_{li}_{h}")
                      with nc.allow_low_precision(reason="f32r rounding ok"):
                          nc.vector.reciprocal(rcp[:, :], ps_o[0:1, 256:512])
                      nc.tensor.matmul(
                          ps_o[0:64, 512:768], ones[0:1, 0:64], rcp[:, :],
                          start=True, stop=True)
                      rep = sbs.tile([128, 256], f32, tag="rep", name=f"rep_{li}_{h}", bufs=2)
                      nc.vector.tensor_copy(rep[0:64, :], ps_o[0:64, 512:768])
                      nc.vector.tensor_mul(attnT[lo:hi, hp, :],
                                           ps_o[0:64, 0:256],
                                           rep[0:64, :])

                  mark(f"L{li}.wo")
                  # ---------------- Wo + residual + LN1 ----------------------
                  ps_y = psp.tile([128, 2048], f32, tag="ps", name=f"psy_{li}")
                  for hp in range(8):
                      wot = sbw.tile([128, 1024], f32r, tag="w", name=f"wo_{li}_{hp}")
                      nc.sync.dma_start(out=wot[:, :], in_=wo_in[li, hp])
                      for ei in range(8):
                          nc.tensor.matmul(
                              ps_y[:, ei * 256:(ei + 1) * 256],
                              wot[:, ei * 128:(ei + 1) * 128],
                              attnT[:, hp, :], start=hp == 0 and ei % 2 == 0,
                              stop=hp == 7)
                  if DBG and li == 0:
                      nc.sync.dma_start(out=dbg_at[:, :, :], in_=attnT[:, :, :].bitcast(f32))
                  resid = sba.tile([128, 8, 256], f32r, tag="resid", bufs=1, name=f"res1_{li}")
                  for ei in range(8):
                      nc.vector.scalar_tensor_tensor(
                          out=resid[:, ei, :], in0=ps_y[:, ei * 256:(ei + 1) * 256],
                          scalar=0.0, in1=xT[:, ei, :].bitcast(f32),
                          op0=ALU.add, op1=ALU.add)

                  if DBG and li == 0:
                      nc.sync.dma_start(out=dbg_r1[:, :, :], in_=resid[:, :, :].bitcast(f32))
                  xT = sbx.tile([128, 8, LC], f32r, tag="xT", name=f"xT_{li}a")
                  layer_norm(resid, xT, g1r_in[li], b1r_in[li], f"{li}a")
                  if DBG and li == 0:
                      nc.sync.dma_start(out=dbg_x1[:, :, :], in_=xT[:, :, :].bitcast(f32))

                  mark(f"L{li}.ffn1")
                  # ---------------- FFN --------------------------------------
                  hT = sbh.tile([128, 32, 256], f32r, tag="hT", name=f"hT_{li}")
                  b1c = sbs.tile([128, 32], f32, tag="b1c", name=f"b1c_{li}")
                  nc.sync.dma_start(out=b1c[:, :], in_=b1c_in[li].rearrange("j p -> p j"))
                  for jg in range(4):
                      ps_h = psp.tile([128, 2048], f32, tag="ps", name=f"psh_{li}_{jg}")
                      for di in range(8):
                          w1t = sbw.tile([128, 8, 128], f32r, tag="w",
                                         name=f"w1_{li}_{jg}_{di}")
                          nc.sync.dma_start(out=w1t[:, :, :], in_=w1_in[li, jg, di])
                          for jj in range(8):
                              nc.tensor.matmul(
                                  ps_h[:, jj * 256:(jj + 1) * 256],
                                  w1t[:, jj, :],
                                  xT[:, di, :], start=di == 0 and jj % 2 == 0,
                                  stop=di == 7)
                      for jj in range(8):
                          nc.scalar.activation(
                              hT[:, jg * 8 + jj, :], ps_h[:, jj * 256:(jj + 1) * 256],
                              AF.Relu, bias=b1c[:, jg * 8 + jj:jg * 8 + jj + 1], scale=1.0)

                  mark(f"L{li}.ffn2")
                  ps_f = psp.tile([128, 2048], f32, tag="ps", name=f"psf_{li}")
                  for jc in range(32):
                      w2t = sbw.tile([128, 1024], f32r, tag="w", name=f"w2_{li}_{jc}")
                      nc.sync.dma_start(out=w2t[:, :], in_=w2_in[li, jc])
                      for ei in range(8):
                          nc.tensor.matmul(
                              ps_f[:, ei * 256:(ei + 1) * 256],
                              w2t[:, ei * 128:(ei + 1) * 128],
                              hT[:, jc, :], start=jc == 0 and ei % 2 == 0,
                              stop=jc == 31)
                  b2c = sbs.tile([128, 8], f32, tag="b2c", name=f"b2c_{li}")
                  nc.sync.dma_start(out=b2c[:, :], in_=b2c_in[li].rearrange("e p -> p e"))
                  resid2 = sba.tile([128, 8, 256], f32r, tag="resid", bufs=1, name=f"res2_{li}")
                  for ei in range(8):
                      nc.vector.scalar_tensor_tensor(
                          out=resid2[:, ei, :], in0=ps_f[:, ei * 256:(ei + 1) * 256],
                          scalar=b2c[:, ei:ei + 1], in1=xT[:, ei, :].bitcast(f32),
                          op0=ALU.add, op1=ALU.add)

                  mark(f"L{li}.ln2")
                  xT = sbx.tile([128, 8, LC], f32r, tag="xT", name=f"xT_{li}b")
                  layer_norm(resid2, xT, g2r_in[li], b2r_in[li], f"{li}b")

            mark("out")
            # ---------------- output: transpose back --------------------
            for lj in range(2):
                ps_t = psp.tile([128, 2048], f32, tag="ps", name=f"pst_{lj}")
                for ei in range(8):
                    nc.tensor.transpose(
                        ps_t[:, ei * 256:ei * 256 + 128],
                        xT[:, ei, lj * 128:(lj + 1) * 128].bitcast(f32),
                        ident[:, :])
                outp = sbs.tile([128, 1024], f32, tag="outp", bufs=1,
                                name=f"outp_{lj}")
                nc.vector.tensor_copy(
                    outp[:, :].rearrange("p (e m) -> p e m", e=8),
                    ps_t[:, :].rearrange("p (e m) -> p e m", e=8)[:, :, 0:128])
                nc.sync.dma_start(out=out_par[lj * 128:(lj + 1) * 128, :],
                                  in_=outp[:, :])
    nc.finalize()
    return nc


def _host_prep(inputs, n_layers):
    """Host-side preprocessing: embedding+PE, weight reshapes, per-core maps."""
    tokens = np.asarray(inputs["tokens"])
    mask = np.asarray(inputs["self_attn_mask"])
    emb = np.asarray(inputs["emb"], dtype=np.float32)
    Wq = np.asarray(inputs["Wq"], dtype=np.float32)
    Wk = np.asarray(inputs["Wk"], dtype=np.float32)
    Wo = np.asarray(inputs["Wo"], dtype=np.float32)
    W1 = np.asarray(inputs["W1"], dtype=np.float32)
    b1 = np.asarray(inputs["b1"], dtype=np.float32)
    W2 = np.asarray(inputs["W2"], dtype=np.float32)
    b2 = np.asarray(inputs["b2"], dtype=np.float32)
    g1 = np.asarray(inputs["ln1_g"], dtype=np.float32)
    be1 = np.asarray(inputs["ln1_b"], dtype=np.float32)
    g2 = np.asarray(inputs["ln2_g"], dtype=np.float32)
    be2 = np.asarray(inputs["ln2_b"], dtype=np.float32)

    # input block (exact f32, same ops as reference)
    emb0 = emb.copy()
    emb0[PAD] = 0.0
    x = emb0[tokens] * np.float32(D ** 0.5)
    pos = np.arange(L, dtype=np.float32)[:, None]
    i = np.arange(D // 2, dtype=np.float32)[None, :]
    angle = pos / (10000.0 ** (2.0 * i / D))
    pe = np.zeros((L, D), np.float32)
    pe[:, 0::2] = np.sin(angle)
    pe[:, 1::2] = np.cos(angle)
    x = (x + pe[None]).astype(np.float32)  # [B, L, D]

    # head-major permutation: new col r = h*64+d'  <- old col d'*H + h
    r = np.arange(D)
    perm = (r % HD) * H + (r // HD)
    Wq_p = np.ascontiguousarray(Wq[:n_layers][:, :, perm])
    Wk_p = np.ascontiguousarray(Wk[:n_layers][:, :, perm])
    Wo_p = np.ascontiguousarray(Wo[:n_layers][:, perm, :])

    wq_d = Wq_p.reshape(n_layers, 8, 128, 1024)
    wk_d = Wk_p.reshape(n_layers, 8, 128, 1024)
    wo_d = Wo_p.reshape(n_layers, 8, 128, 1024)
    w1_d = np.ascontiguousarray(
        W1[:n_layers].reshape(n_layers, 8, 128, 4, 8, 128).transpose(0, 3, 1, 2, 4, 5))
    w2_d = np.ascontiguousarray(W2[:n_layers].reshape(n_layers, 32, 128, 1024))
    b1c = b1[:n_layers].reshape(n_layers, 32, 128)
    b2c = b2[:n_layers].reshape(n_layers, 8, 128)
    g1r = g1[:n_layers].reshape(n_layers, 8, 128)
    b1r = be1[:n_layers].reshape(n_layers, 8, 128)
    g2r = g2[:n_layers].reshape(n_layers, 8, 128)
    b2r = be2[:n_layers].reshape(n_layers, 8, 128)
    ones = np.ones((128, 256), np.float32)

    shared = dict(wq=wq_d, wk=wk_d, wo=wo_d, w1=w1_d, w2=w2_d,
                  b1c=b1c, b2c=b2c, g1r=g1r, b1r=b1r, g2r=g2r, b2r=b2r,
                  ones=ones)

    in_maps = []
    for c in range(NCORES):
        b = c // 4
        j = c % 4
        xT0 = np.ascontiguousarray(
            x[b, j * LC:(j + 1) * LC, :].T).reshape(8, 128, LC)
        keep = (~mask[b, 0, :, 0]).astype(np.float32)  # 1.0 where key kept
        km = keep.reshape(8, 128)
        kmo = keep[j * LC:(j + 1) * LC].reshape(2, 128)
        m = dict(shared)
        m.update(xT_in=xT0, km=km, kmo=kmo)
        in_maps.append(m)
    return x, in_maps


def kernel(**inputs) -> np.ndarray:
    from concourse.bass_utils import run_bass_kernel_spmd

    n_layers = _DEV_NL
    if "nc" not in _cache or _cache.get("nl") != n_layers:
        _cache["nc"] = _build_nc(n_layers)
        _cache["nl"] = n_layers
    nc = _cache["nc"]

    _, in_maps = _host_prep(inputs, n_layers)
    res = run_bass_kernel_spmd(nc, in_maps, core_ids=list(range(NCORES)))
    out = np.empty((B, L, D), np.float32)
    for c in range(NCORES):
        b, j = c // 4, c % 4
        out[b, j * LC:(j + 1) * LC, :] = res.results[c]["out"]
    return out



# revision 8
# speedup vs baseline: 1.8663x; 1.8663x over previous
"""Trainium2 Bass kernel for nn_Encoder (4-layer dense transformer encoder).

Sharding: sequence-sharded data parallel. 8 cores = 2 batches x 4 sequence
chunks of 256 tokens. Per layer each core computes its own K chunk in natural
layout (keys as partitions), masks it, appends the 0/1 keep-mask as a 65th
column per head, and AllGathers it within its batch's 4-core group. The
transposed-K layout needed for scores is rebuilt locally with PE transposes.
Activations stay transposed (xT [D, L_local]); weights stream from HBM in
bfloat16; matmuls mix bf16 weights with f32r activations (PSUM accumulates
f32).

The attention key mask is applied multiplicatively: rows of kn for masked keys
are zeroed and the softmax denominator comes for free from the keep-mask
column (row 64 of the o-matmul output), numerically identical to the
reference's where(mask, -1e9, score) followed by softmax.

Self-contained: hardcodes all shapes; host side does the embedding gather,
positional encoding, weight permutations/reshapes, and output assembly.
"""
import os
import numpy as np

B, L, D, H, M, NL, V = 2, 1024, 1024, 16, 4096, 4, 32000
HD = D // H          # 64
LC = 256             # local sequence chunk per core
NCORES = 8
PAD = 0

_DEV_NL = int(os.environ.get("KERNEL_NL", str(NL)))

_cache = {}
PHASE_LOG = []


def _build_nc(n_layers, reps=1):
    import os
    import contextlib
    import concourse.mybir as mybir
    import concourse.tile as tile
    from concourse import bacc
    from concourse.masks import make_identity

    f32 = mybir.dt.float32
    f32r = mybir.dt.float32r
    bf16 = mybir.dt.bfloat16
    AF = mybir.ActivationFunctionType
    ALU = mybir.AluOpType

    FAKE_AG = os.environ.get("KERNEL_FAKE_AG", "0") == "1"
    nc = bacc.Bacc(None, target_bir_lowering=False, num_devices=NCORES)
    PHASE_LOG.clear()

    def mark(label):
        PHASE_LOG.append((label, len(nc.inst_map)))

    def par(name, shape, dt, out=False):
        return nc.declare_dram_parameter(name, list(shape), dt, isOutput=out)

    xT_in = par("xT_in", [8, 128, LC], bf16)
    wq_in = par("wq", [n_layers, 8, 128, 1024], bf16)   # [li][di][p][head-major col]
    wk_in = par("wk", [n_layers, 8, 128, 1024], bf16)
    wo_in = par("wo", [n_layers, 8, 128, 1024], bf16)   # [li][hp][p][e]
    w1_in = par("w1", [n_layers, 4, 8, 128, 1024], bf16)  # [li][jg][di][p][(jj q)]
    w2_in = par("w2", [n_layers, 32, 128, 1024], bf16)  # [li][jc][p][e]
    b1c_in = par("b1c", [n_layers, 32, 128], f32)
    b2c_in = par("b2c", [n_layers, 8, 128], f32)
    g1r_in = par("g1r", [n_layers, 8, 128], f32r)
    b1r_in = par("b1r", [n_layers, 8, 128], f32r)
    g2r_in = par("g2r", [n_layers, 8, 128], f32r)
    b2r_in = par("b2r", [n_layers, 8, 128], f32r)
    kmo_in = par("kmo", [2, 128], f32r)                 # keep-mask for own 256 keys
    ones_in = par("ones", [128, 256], f32r)
    out_par = par("out", [LC, D], f32, out=True)

    EPS = 1e-5
    INV_D = 1.0 / float(D)
    INV_SQ = 0.125  # 1/sqrt(HD)

    with tile.TileContext(nc) as tc:
        ctx = contextlib.ExitStack()
        with ctx:
            sbc = ctx.enter_context(tc.tile_pool(name="const", bufs=1))
            sbx = ctx.enter_context(tc.tile_pool(name="xt", bufs=2))
            sbk = ctx.enter_context(tc.tile_pool(name="kbuf", bufs=1))
            sbq = ctx.enter_context(tc.tile_pool(name="qbuf", bufs=1))
            sbw = ctx.enter_context(tc.tile_pool(name="wts", bufs=24))
            sba = ctx.enter_context(tc.tile_pool(name="act", bufs=2))
            sbh = ctx.enter_context(tc.tile_pool(name="hbuf", bufs=1))
            sbs = ctx.enter_context(tc.tile_pool(name="small", bufs=4))
            psA = ctx.enter_context(tc.tile_pool(name="psA", bufs=2, space="PSUM"))
            psB = ctx.enter_context(tc.tile_pool(name="psB", bufs=2, space="PSUM"))
            drp = ctx.enter_context(tc.tile_pool(name="dram", bufs=2, space="DRAM"))

            ones = sbc.tile([128, 256], f32r, name="ones_t")
            nc.sync.dma_start(out=ones[:, :], in_=ones_in[:, :])
            kmo = sbc.tile([128, 2], f32r, name="kmo_t")
            nc.sync.dma_start(out=kmo[:, :], in_=kmo_in.rearrange("m p -> p m"))
            ident = sbc.tile([128, 128], f32, name="ident_t")
            make_identity(nc, ident[:, :])
            identb = sbc.tile([128, 128], bf16, name="identb_t")
            make_identity(nc, identb[:, :])

            xT = sbx.tile([128, 8, LC], bf16, tag="xT", name="xT0")
            nc.sync.dma_start(out=xT[:, :, :], in_=xT_in.rearrange("e p l -> p e l"))

            def layer_norm(resid, xT_out, gr_dram, br_dram, uid):
                """xT_out = LN(resid) * g + b, per-column-l stats."""
                grt = sbs.tile([1, 8, 128], f32r, tag="gr", bufs=2, name=f"gr_{uid}")
                nc.sync.dma_start(out=grt[:, :, :], in_=gr_dram.unsqueeze(0))
                brt = sbs.tile([1, 8, 128], f32r, tag="br", bufs=2, name=f"br_{uid}")
                nc.sync.dma_start(out=brt[:, :, :], in_=br_dram.unsqueeze(0))

                ps_st = psB.tile([128, 1024], f32, tag="ps", name=f"psst_{uid}")
                for ei in range(8):
                    sq1 = sbs.tile([128, 256], f32r, tag="sqtmp", bufs=2,
                                   name=f"sq_{uid}_{ei}")
                    nc.scalar.activation(sq1[:, :], resid[:, ei, :].bitcast(f32),
                                         AF.Square)
                    nc.tensor.matmul(ps_st[0:1, 0:256], ones[:, 0:1],
                                     resid[:, ei, :], start=ei == 0, stop=False)
                    nc.tensor.matmul(ps_st[0:1, 256:512], ones[:, 0:1],
                                     sq1[:, :], start=False, stop=ei == 7)
                mu = sbs.tile([1, 256], f32, tag="st1", bufs=1, name=f"mu_{uid}")
                nc.vector.tensor_scalar_mul(mu[:, :], ps_st[0:1, 0:256], INV_D)
                ex2 = sbs.tile([1, 256], f32, tag="st2", bufs=1, name=f"ex2_{uid}")
                nc.vector.tensor_scalar_mul(ex2[:, :], ps_st[0:1, 256:512], INV_D)
                mu2 = sbs.tile([1, 256], f32, tag="st3", bufs=1, name=f"mu2_{uid}")
                nc.vector.tensor_mul(mu2[:, :], mu[:, :], mu[:, :])
                var = sbs.tile([1, 256], f32, tag="st4", bufs=1, name=f"var_{uid}")
                nc.vector.scalar_tensor_tensor(
                    out=var[:, :], in0=ex2[:, :], scalar=EPS, in1=mu2[:, :],
                    op0=ALU.add, op1=ALU.subtract)
                sd = sbs.tile([1, 256], f32, tag="st5", bufs=1, name=f"sd_{uid}")
                nc.scalar.activation(sd[:, :], var[:, :], AF.Sqrt)
                rstd = sbs.tile([1, 256], f32r, tag="st6", bufs=1, name=f"rstd_{uid}")
                with nc.allow_low_precision(reason="f32r rounding ok"):
                    nc.vector.reciprocal(rstd[:, :], sd[:, :])
                nmr = sbs.tile([1, 256], f32r, tag="st7", bufs=1, name=f"nmr_{uid}")
                nc.vector.scalar_tensor_tensor(
                    out=nmr[:, :], in0=mu[:, :], scalar=-1.0, in1=rstd[:, :].bitcast(f32),
                    op0=ALU.mult, op1=ALU.mult)
                # per pair of ei: rg/nb in one [128,1024] tile (2 banks), then
                # two 512-free DVE applies
                for pair in range(4):
                    e0 = pair * 2
                    ps_rn = psB.tile([128, 1024], f32, tag="ps", name=f"psrn_{uid}_{pair}")
                    for k in range(2):
                        ei = e0 + k
                        base = k * 512
                        nc.tensor.matmul(ps_rn[:, base:base + 256],
                                         grt[0:1, ei, :], rstd[:, :],
                                         start=True, stop=False)
                        nc.tensor.matmul(ps_rn[:, base + 256:base + 512],
                                         grt[0:1, ei, :], nmr[:, :],
                                         start=False, stop=False)
                        nc.tensor.matmul(ps_rn[:, base + 256:base + 512],
                                         brt[0:1, ei, :], ones[0:1, 0:256],
                                         start=False, stop=True)
                    rn = ps_rn[:, :].rearrange("p (e c) -> p e c", e=2)
                    tmp = sbs.tile([128, 512], f32, tag="lntmp", bufs=2,
                                   name=f"lnt_{uid}_{pair}")
                    nc.vector.tensor_mul(tmp[:, :].rearrange("p (e c) -> p e c", e=2),
                                         resid[:, e0:e0 + 2, :].bitcast(f32),
                                         rn[:, :, 0:256])
                    nc.vector.tensor_tensor(
                        out=xT_out[:, e0:e0 + 2, :],
                        in0=tmp[:, :].rearrange("p (e c) -> p e c", e=2),
                        in1=rn[:, :, 256:512], op=ALU.add)

            for rep in range(reps):
              for li in range(n_layers):
                  # ---------------- K phase: own-chunk kn (natural) ----------
                  mark(f"L{li}.k")
                  psK = [psA.tile([128, 1024], f32, tag="ps", name=f"psk_{li}_{ms}")
                         for ms in range(2)]
                  for di in range(8):
                      wkt = sbw.tile([128, 1024], bf16, tag="w", name=f"wk_{li}_{di}")
                      nc.sync.dma_start(out=wkt[:, :], in_=wk_in[li, di])
                      for ms in range(2):
                          for nn_ in range(2):
                              nc.tensor.matmul(
                                  psK[ms][:, nn_ * 512:(nn_ + 1) * 512],
                                  xT[:, di, ms * 128:(ms + 1) * 128],
                                  wkt[:, nn_ * 512:(nn_ + 1) * 512],
                                  start=di == 0, stop=di == 7)
                  # mask keys + append keep-mask as 65th column per head
                  knc = sbk.tile([128, 2, 1040], bf16, tag="knc", bufs=2,
                                 name=f"knc_{li}")
                  for ms in range(2):
                      nc.vector.tensor_scalar_mul(
                          knc[:, ms].rearrange("p (h c) -> p h c", h=16)[:, :, 0:64],
                          psK[ms][:, :].rearrange("p (h c) -> p h c", h=16),
                          kmo[:, ms:ms + 1].bitcast(f32))
                      nc.vector.tensor_scalar_mul(
                          knc[:, ms].rearrange("p (h c) -> p h c", h=16)[:, :, 64:65],
                          ones[:, 0:16].unsqueeze(2).bitcast(f32),
                          kmo[:, ms:ms + 1].bitcast(f32))

                  # AllGather kn only (keys 1040-col blocks)
                  agi = drp.tile([8, 128, 260], bf16, tag="agi", name=f"agi_{li}")
                  ago = drp.tile([4, 8, 128, 260], bf16, tag="ago", name=f"ago_{li}")
                  nc.sync.dma_start(
                      out=agi.rearrange("b p c -> p b c"),
                      in_=knc[:, :, :].rearrange("p m (g c) -> p (m g) c", g=4))
                  if FAKE_AG:
                      for r in range(4):
                          nc.sync.dma_start(out=ago[r], in_=agi[:, :, :])
                  else:
                      nc.gpsimd.collective_compute(
                          "AllGather", mybir.AluOpType.bypass,
                          replica_groups=[[0, 1, 2, 3], [4, 5, 6, 7]],
                          ins=[agi.opt()], outs=[ago.opt()])
                  kn = sbk.tile([128, 8, 1040], bf16, tag="kn", bufs=1,
                                name=f"kn_{li}")
                  for r in range(4):
                      nc.sync.dma_start(
                          out=kn[:, r * 2:(r + 1) * 2, :].rearrange(
                              "p m (g c) -> p m g c", g=4),
                          in_=ago[r].rearrange("(m g) p c -> p m g c", m=2))

                  # ---------------- Q phase (overlaps AG) --------------------
                  mark(f"L{li}.q")
                  psQ = [psB.tile([128, 1024], f32, tag="ps", name=f"psq_{li}_{j}")
                         for j in range(2)]
                  for di in range(8):
                      wqt = sbw.tile([128, 1024], bf16, tag="w", name=f"wq_{li}_{di}")
                      nc.sync.dma_start(out=wqt[:, :], in_=wq_in[li, di])
                      for hp in range(8):
                          nc.tensor.matmul(
                              psQ[hp // 4][:, (hp % 4) * 256:(hp % 4 + 1) * 256],
                              wqt[:, hp * 128:(hp + 1) * 128],
                              xT[:, di, :],
                              start=di == 0 and hp % 2 == 0,
                              stop=di == 7 and hp % 2 == 1)
                  qT = sbq.tile([128, 8, 256], bf16, tag="qT", name=f"qT_{li}")
                  for j in range(2):
                      nc.scalar.copy(
                          qT[:, 4 * j:4 * j + 4, :],
                          psQ[j][:, :].rearrange("p (i m) -> p i m", i=4))

                  # ---------------- kT via PE transposes of gathered kn ------
                  mark(f"L{li}.t")
                  kT = sbk.tile([128, 8, 1024], bf16, tag="kT", bufs=1,
                                name=f"kT_{li}")
                  for hh in range(4):   # two hp per psum tile
                      psT = psA.tile([128, 2048], bf16, tag="ps", name=f"pst_{li}_{hh}")
                      for k in range(2):
                          hp = hh * 2 + k
                          for mi in range(8):
                              for sub in range(2):
                                  h = 2 * hp + sub
                                  nc.tensor.transpose(
                                      psT[sub * 64:(sub + 1) * 64,
                                          k * 1024 + mi * 128:k * 1024 + (mi + 1) * 128],
                                      kn[:, mi, h * 65:h * 65 + 64],
                                      identb[:, :])
                      if hh % 2 == 0:
                          nc.vector.tensor_copy(
                              kT[:, 2 * hh:2 * hh + 2, :].rearrange(
                                  "p a m -> p (a m)"),
                              psT[:, :])
                      else:
                          nc.scalar.copy(
                              kT[:, 2 * hh:2 * hh + 2, :].rearrange(
                                  "p a m -> p (a m)"),
                              psT[:, :])

                  # ---------------- attention, head by head ------------------
                  mark(f"L{li}.attn")
                  attnT = sba.tile([128, 8, 256], bf16, tag="attnT", bufs=1,
                                   name=f"attnT_{li}")
                  for h in range(H):
                      hp, sub = h // 2, h % 2
                      lo, hi = sub * 64, sub * 64 + 64
                      wT = sba.tile([128, 8, 256], bf16, tag="wT", bufs=2,
                                    name=f"wT_{li}_{h}")
                      for half in range(2):
                          ps_s = psA.tile([128, 1024], f32, tag="ps",
                                          name=f"pss_{li}_{h}_{half}")
                          for m4 in range(4):
                              mi = half * 4 + m4
                              nc.tensor.matmul(
                                  ps_s[:, m4 * 256:(m4 + 1) * 256],
                                  kT[lo:hi, hp, mi * 128:(mi + 1) * 128],
                                  qT[lo:hi, hp, :],
                                  start=m4 % 2 == 0, stop=m4 % 2 == 1)
                          nc.scalar.activation(
                              wT[:, half * 4:(half + 1) * 4, :],
                              ps_s[:, :].rearrange("p (i m) -> p i m", i=4),
                              AF.Exp, bias=0.0, scale=INV_SQ)
                      ps_o = psB.tile([128, 1024], f32, tag="ps",
                                      name=f"pso_{li}_{h}")
                      for mi in range(8):
                          nc.tensor.matmul(
                              ps_o[0:65, 0:256],
                              kn[:, mi, h * 65:(h + 1) * 65],
                              wT[:, mi, :], start=mi == 0, stop=mi == 7)
                      rcp = sbs.tile([1, 256], f32r, tag="rcp", bufs=2,
                                     name=f"rcp_{li}_{h}")
                      with nc.allow_low_precision(reason="f32r rounding ok"):
                          nc.vector.reciprocal(rcp[:, :], ps_o[64:65, 0:256])
                      nc.tensor.matmul(
                          ps_o[0:64, 512:768], ones[0:1, 0:64], rcp[:, :],
                          start=True, stop=True)
                      rep = sbs.tile([128, 256], f32, tag="rep", bufs=2,
                                     name=f"rep_{li}_{h}")
                      nc.vector.tensor_copy(rep[0:64, :], ps_o[0:64, 512:768])
                      nc.vector.tensor_mul(attnT[lo:hi, hp, :],
                                           ps_o[0:64, 0:256],
                                           rep[0:64, :])

                  # ---------------- Wo + residual + LN1 ----------------------
                  mark(f"L{li}.wo")
                  psY = [psA.tile([128, 1024], f32, tag="ps", name=f"psy_{li}_{j}")
                         for j in range(2)]
                  for hp in range(8):
                      wot = sbw.tile([128, 1024], bf16, tag="w", name=f"wo_{li}_{hp}")
                      nc.sync.dma_start(out=wot[:, :], in_=wo_in[li, hp])
                      for ei in range(8):
                          nc.tensor.matmul(
                              psY[ei // 4][:, (ei % 4) * 256:(ei % 4 + 1) * 256],
                              wot[:, ei * 128:(ei + 1) * 128],
                              attnT[:, hp, :],
                              start=hp == 0 and ei % 2 == 0,
                              stop=hp == 7 and ei % 2 == 1)
                  resid = sba.tile([128, 8, 256], f32r, tag="resid", bufs=2,
                                   name=f"res1_{li}")
                  for j in range(2):
                      nc.vector.scalar_tensor_tensor(
                          out=resid[:, 4 * j:4 * j + 4, :],
                          in0=psY[j][:, :].rearrange("p (i m) -> p i m", i=4),
                          scalar=0.0, in1=xT[:, 4 * j:4 * j + 4, :],
                          op0=ALU.add, op1=ALU.add)

                  xT = sbx.tile([128, 8, LC], bf16, tag="xT", name=f"xT_{li}a")
                  layer_norm(resid, xT, g1r_in[li], b1r_in[li], f"{li}a")

                  # ---------------- FFN --------------------------------------
                  mark(f"L{li}.ffn1")
                  hT = sbh.tile([128, 32, 256], bf16, tag="hT", name=f"hT_{li}")
                  b1c = sbs.tile([128, 32], f32, tag="b1c", name=f"b1c_{li}")
                  nc.sync.dma_start(out=b1c[:, :], in_=b1c_in[li].rearrange("j p -> p j"))
                  for jg in range(4):
                      pool = psA if jg % 2 == 0 else psB
                      psH = [pool.tile([128, 1024], f32, tag="ps",
                                       name=f"psh_{li}_{jg}_{j}") for j in range(2)]
                      for di in range(8):
                          w1t = sbw.tile([128, 8, 128], bf16, tag="w",
                                         name=f"w1_{li}_{jg}_{di}")
                          nc.sync.dma_start(
                              out=w1t[:, :, :],
                              in_=w1_in[li, jg, di].rearrange("p (j q) -> p j q", j=8))
                          for jj in range(8):
                              nc.tensor.matmul(
                                  psH[jj // 4][:, (jj % 4) * 256:(jj % 4 + 1) * 256],
                                  w1t[:, jj, :],
                                  xT[:, di, :],
                                  start=di == 0 and jj % 2 == 0,
                                  stop=di == 7 and jj % 2 == 1)
                      for jj in range(8):
                          nc.scalar.activation(
                              hT[:, jg * 8 + jj, :],
                              psH[jj // 4][:, (jj % 4) * 256:(jj % 4 + 1) * 256],
                              AF.Relu, bias=b1c[:, jg * 8 + jj:jg * 8 + jj + 1],
                              scale=1.0)

                  mark(f"L{li}.ffn2")
                  psF = [psA.tile([128, 1024], f32, tag="ps", name=f"psf_{li}_{j}")
                         for j in range(2)]
                  for jc in range(32):
                      w2t = sbw.tile([128, 1024], bf16, tag="w", name=f"w2_{li}_{jc}")
                      nc.sync.dma_start(out=w2t[:, :], in_=w2_in[li, jc])
                      for ei in range(8):
                          nc.tensor.matmul(
                              psF[ei // 4][:, (ei % 4) * 256:(ei % 4 + 1) * 256],
                              w2t[:, ei * 128:(ei + 1) * 128],
                              hT[:, jc, :],
                              start=jc == 0 and ei % 2 == 0,
                              stop=jc == 31 and ei % 2 == 1)
                  b2c = sbs.tile([128, 8], f32, tag="b2c", name=f"b2c_{li}")
                  nc.sync.dma_start(out=b2c[:, :], in_=b2c_in[li].rearrange("e p -> p e"))
                  resid2 = sba.tile([128, 8, 256], f32r, tag="resid", bufs=2,
                                    name=f"res2_{li}")
                  for ei in range(8):
                      nc.vector.scalar_tensor_tensor(
                          out=resid2[:, ei, :],
                          in0=psF[ei // 4][:, (ei % 4) * 256:(ei % 4 + 1) * 256],
                          scalar=b2c[:, ei:ei + 1], in1=xT[:, ei, :],
                          op0=ALU.add, op1=ALU.add)

                  mark(f"L{li}.ln2")
                  xT = sbx.tile([128, 8, LC], bf16, tag="xT", name=f"xT_{li}b")
                  layer_norm(resid2, xT, g2r_in[li], b2r_in[li], f"{li}b")

            mark("out")
            # ---------------- output: transpose back --------------------
            for lj in range(2):
                ps_t = psA.tile([128, 1024], bf16, tag="ps", name=f"psout_{lj}")
                for ei in range(8):
                    nc.tensor.transpose(
                        ps_t[:, ei * 128:(ei + 1) * 128],
                        xT[:, ei, lj * 128:(lj + 1) * 128],
                        identb[:, :])
                outp = sbs.tile([128, 1024], f32, tag="outp", bufs=2,
                                name=f"outp_{lj}")
                nc.vector.tensor_copy(outp[:, :], ps_t[:, :])
                nc.sync.dma_start(out=out_par[lj * 128:(lj + 1) * 128, :],
                                  in_=outp[:, :])
    nc.finalize()
    return nc


def _host_prep(inputs, n_layers):
    """Host-side preprocessing: embedding+PE, weight reshapes, per-core maps."""
    import ml_dtypes
    bf16 = ml_dtypes.bfloat16
    tokens = np.asarray(inputs["tokens"])
    mask = np.asarray(inputs["self_attn_mask"])
    emb = np.asarray(inputs["emb"], dtype=np.float32)
    Wq = np.asarray(inputs["Wq"], dtype=np.float32)
    Wk = np.asarray(inputs["Wk"], dtype=np.float32)
    Wo = np.asarray(inputs["Wo"], dtype=np.float32)
    W1 = np.asarray(inputs["W1"], dtype=np.float32)
    b1 = np.asarray(inputs["b1"], dtype=np.float32)
    W2 = np.asarray(inputs["W2"], dtype=np.float32)
    b2 = np.asarray(inputs["b2"], dtype=np.float32)
    g1 = np.asarray(inputs["ln1_g"], dtype=np.float32)
    be1 = np.asarray(inputs["ln1_b"], dtype=np.float32)
    g2 = np.asarray(inputs["ln2_g"], dtype=np.float32)
    be2 = np.asarray(inputs["ln2_b"], dtype=np.float32)

    # input block (exact f32, same ops as reference)
    emb0 = emb.copy()
    emb0[PAD] = 0.0
    x = emb0[tokens] * np.float32(D ** 0.5)
    pos = np.arange(L, dtype=np.float32)[:, None]
    i = np.arange(D // 2, dtype=np.float32)[None, :]
    angle = pos / (10000.0 ** (2.0 * i / D))
    pe = np.zeros((L, D), np.float32)
    pe[:, 0::2] = np.sin(angle)
    pe[:, 1::2] = np.cos(angle)
    x = (x + pe[None]).astype(np.float32)  # [B, L, D]

    # head-major permutation: new col r = h*64+d'  <- old col d'*H + h
    r = np.arange(D)
    perm = (r % HD) * H + (r // HD)
    Wq_p = np.ascontiguousarray(Wq[:n_layers][:, :, perm])
    Wk_p = np.ascontiguousarray(Wk[:n_layers][:, :, perm])
    Wo_p = np.ascontiguousarray(Wo[:n_layers][:, perm, :])

    wq_d = Wq_p.reshape(n_layers, 8, 128, 1024).astype(bf16)
    wk_d = Wk_p.reshape(n_layers, 8, 128, 1024).astype(bf16)
    wo_d = Wo_p.reshape(n_layers, 8, 128, 1024).astype(bf16)
    w1_d = np.ascontiguousarray(
        W1[:n_layers].reshape(n_layers, 8, 128, 4, 8, 128).transpose(
            0, 3, 1, 2, 4, 5).reshape(n_layers, 4, 8, 128, 1024)).astype(bf16)
    w2_d = np.ascontiguousarray(
        W2[:n_layers].reshape(n_layers, 32, 128, 1024)).astype(bf16)
    b1c = b1[:n_layers].reshape(n_layers, 32, 128)
    b2c = b2[:n_layers].reshape(n_layers, 8, 128)
    g1r = g1[:n_layers].reshape(n_layers, 8, 128)
    b1r = be1[:n_layers].reshape(n_layers, 8, 128)
    g2r = g2[:n_layers].reshape(n_layers, 8, 128)
    b2r = be2[:n_layers].reshape(n_layers, 8, 128)
    ones = np.ones((128, 256), np.float32)

    shared = dict(wq=wq_d, wk=wk_d, wo=wo_d, w1=w1_d, w2=w2_d,
                  b1c=b1c, b2c=b2c, g1r=g1r, b1r=b1r, g2r=g2r, b2r=b2r,
                  ones=ones)

    in_maps = []
    for c in range(NCORES):
        b = c // 4
        j = c % 4
        xT0 = np.ascontiguousarray(
            x[b, j * LC:(j + 1) * LC, :].T).reshape(8, 128, LC).astype(bf16)
        keep = (~mask[b, 0, :, 0]).astype(np.float32)  # 1.0 where key kept
        kmo = keep[j * LC:(j + 1) * LC].reshape(2, 128)
        m = dict(shared)
        m.update(xT_in=xT0, kmo=kmo)
        in_maps.append(m)
    return x, in_maps


def kernel(**inputs) -> np.ndarray:
    from concourse.bass_utils import run_bass_kernel_spmd

    n_layers = _DEV_NL
    if "nc" not in _cache or _cache.get("nl") != n_layers:
        _cache["nc"] = _build_nc(n_layers)
        _cache["nl"] = n_layers
    nc = _cache["nc"]

    _, in_maps = _host_prep(inputs, n_layers)
    res = run_bass_kernel_spmd(nc, in_maps, core_ids=list(range(NCORES)))
    out = np.empty((B, L, D), np.float32)
    for c in range(NCORES):
        b, j = c // 4, c % 4
        out[b, j * LC:(j + 1) * LC, :] = res.results[c]["out"]
    return out
